# revision 18
# baseline (speedup 1.0000x reference)
"""MoE model (B=8,S=2048,H=512,E=8,K=2) on 8 TRN2 NeuronCores.

Strategy: data-parallel over the batch dim (one batch row of 2048 tokens per
core). Each core computes the gate (softmax + top-2 via full top-8 sort),
then iterates over all 8 experts densely: h = gelu(x @ w1[e] + b1[e]),
y = h @ w2[e], accumulating out += wsel[:, e] * y in SBUF, with the b2
contribution folded in as an init matmul out0 = wsel @ b2. All matmuls run
as float32r (full fp32 storage, ~bf16 PE throughput, ~1e-3 accuracy).

Layout notes:
 - xT (transposed x) is built on-device via PE transposes; it feeds both the
   gate matmul and layer-1 (contraction over H needs H on partitions).
 - Layer-1 output h1 is kept transposed [f, t] so layer-2 can use it directly
   as the stationary operand, producing y in natural [t, o] layout where the
   per-token gate weight is a per-partition scalar multiply.
"""

import numpy as np

B, S, H, E = 8, 2048, 512, 8
F = 4 * H          # 2048
T = S              # tokens per core (one batch row per core)
P = 128
HC = H // P        # 4 contraction chunks for layer 1
FC = F // P        # 16 f chunks
TT = T // P        # 16 token tiles
NT = 512           # moving-dim tile (max for 4-byte dtypes)
TS4 = T // NT      # 4 token chunks of 512

_CACHE = {}


def _build(act_name="Gelu"):
    from concourse import bacc
    import concourse.bass as bass
    import concourse.mybir as mybir
    import concourse.tile as tile
    from concourse.masks import make_identity

    ts = bass.ts
    ds = bass.ds
    F32 = mybir.dt.float32
    F32R = mybir.dt.float32r
    AF = mybir.ActivationFunctionType
    OP = mybir.AluOpType

    ACT_FN = getattr(mybir.ActivationFunctionType, act_name)

    nc = bacc.Bacc("TRN2", target_bir_lowering=False)

    x_d = nc.dram_tensor("x", [T, H], F32, kind="ExternalInput")
    wg_d = nc.dram_tensor("W_g", [P * E, H], F32, kind="ExternalInput")
    w1_d = nc.dram_tensor("w1", [E * H, F], F32R, kind="ExternalInput")
    b1_d = nc.dram_tensor("b1", [E * P, FC], F32, kind="ExternalInput")
    w2_d = nc.dram_tensor("w2", [E * F, H], F32R, kind="ExternalInput")
    b2_d = nc.dram_tensor("b2", [E, H], F32, kind="ExternalInput")
    out_d = nc.dram_tensor("out", [T, H], F32, kind="ExternalOutput")
    dlog_d = nc.dram_tensor("dbg_logit", [T, E], F32, kind="ExternalOutput")
    dsel_d = nc.dram_tensor("dbg_wsel", [T, E], F32, kind="ExternalOutput")
    dsrt_d = nc.dram_tensor("dbg_srt", [T, 8], F32, kind="ExternalOutput")

    with tile.TileContext(nc) as tc:
        with tc.tile_pool(name="const", bufs=1) as cpool:
            ident_f = cpool.tile([P, P], F32)
            make_identity(nc, ident_f[:])

            b2sb = cpool.tile([E, H], F32)
            nc.sync.dma_start(b2sb[:], b2_d[:])

            xT = cpool.tile([P, HC, T], F32R)        # x transposed [h, t]
            wsel = cpool.tile([P, TT, E], F32)       # masked gate weights
            out_acc = cpool.tile([P, TT, H], F32)    # running output [t, o]

            # W_g columns broadcast across partitions (host-prepared layout):
            # wgb[p, e, :] = W_g[:, e] for every p. The gate runs as exact-fp32
            # dot products on DVE; the PE's fp32 modes carry ~1e-4 error,
            # enough to flip near-tied top-2 picks.
            wgb = cpool.tile([P, E, H], F32)
            nc.sync.dma_start(wgb[:], wg_d.rearrange("(p c) h -> p c h", p=P))

            # ---- stage 1: load x, transpose into xT, gate ---------------
            with (
                tc.tile_pool(name="xstage", bufs=3) as xpool,
                tc.tile_pool(name="tpsum", bufs=2, space="PSUM") as tpsum,
                tc.tile_pool(name="gate", bufs=4) as gpool,
                tc.tile_pool(name="gsc", bufs=3) as gsc,
                tc.tile_pool(name="wtpsum", bufs=2, space="PSUM") as wtpsum,
                tc.tile_pool(name="bpsum", bufs=2, space="PSUM") as bpsum,
            ):
                for tt in range(TT):
                    xt = xpool.tile([P, H], F32, tag="xt")
                    nc.sync.dma_start(xt[:], x_d[ts(tt, P), :])
                    for hc in range(HC):
                        ps = tpsum.tile([P, P], F32, tag="tp")
                        nc.tensor.transpose(ps[:], xt[:, ts(hc, P)], ident_f[:])
                        # cast-copy rounds to f32r for the expert matmuls
                        nc.vector.tensor_copy(xT[:, hc, ts(tt, P)], ps[:])

                    # gate logits: IEEE-fp32 products on GpSimd (the DVE
                    # fp32 multiplier is ~tf32 and flips near-tied top-2
                    # picks), exact reduce-add on DVE
                    logit = gpool.tile([P, E], F32, tag="logit")
                    for e in range(E):
                        sc = gsc.tile([P, H], F32, tag="gsc")
                        nc.gpsimd.tensor_tensor(
                            out=sc[:], in0=xt[:],
                            in1=wgb[:, e, :], op=OP.mult,
                        )
                        nc.vector.tensor_reduce(
                            out=logit[:, e:e + 1], in_=sc[:],
                            axis=mybir.AxisListType.X, op=OP.add,
                        )
                    srt = gpool.tile([P, 8], F32, tag="srt")
                    nc.vector.max(srt[:], logit[:])
                    nc.sync.dma_start(dlog_d[ts(tt, P), :], logit[:])
                    nc.sync.dma_start(dsrt_d[ts(tt, P), :], srt[:])
                    nmax = gpool.tile([P, 1], F32, tag="nmax")
                    nc.vector.tensor_scalar_mul(nmax[:], srt[:, 0:1], -1.0)
                    expv = gpool.tile([P, E], F32, tag="expv")
                    sume = gpool.tile([P, 1], F32, tag="sume")
                    nc.scalar.activation(
                        expv[:], logit[:], AF.Exp,
                        bias=nmax[:], scale=1.0, accum_out=sume[:],
                    )
                    rsum = gpool.tile([P, 1], F32, tag="rsum")
                    nc.vector.reciprocal(rsum[:], sume[:])
                    probs = gpool.tile([P, E], F32, tag="probs")
                    nc.vector.tensor_scalar_mul(probs[:], expv[:], rsum[:])
                    # wsel = (logit >= 2nd-max) * probs  -> top-2 weights
                    nc.vector.scalar_tensor_tensor(
                        out=wsel[:, tt, :], in0=logit[:], scalar=srt[:, 1:2],
                        in1=probs[:], op0=OP.is_ge, op1=OP.mult,
                    )
                    nc.sync.dma_start(dsel_d[ts(tt, P), :], wsel[:, tt, :])
                    # out_acc[tt] = wsel[tt] @ b2  (the weighted-b2 term)
                    wtp = wtpsum.tile([E, P], F32, tag="wtp")
                    nc.tensor.transpose(wtp[:], wsel[:, tt, :], ident_f[:])
                    wts = gpool.tile([E, P], F32, tag="wts")
                    nc.vector.tensor_copy(wts[:], wtp[:])
                    bp = bpsum.tile([P, H], F32, tag="bp")
                    nc.tensor.matmul(bp[:], wts[:], b2sb[:], start=True, stop=True)
                    nc.vector.tensor_copy(out_acc[:, tt, :], bp[:])

            # ---- stage 3: experts --------------------------------------
            with (
                tc.tile_pool(name="w1p", bufs=5) as w1p,
                tc.tile_pool(name="w2p", bufs=18) as w2p,
                tc.tile_pool(name="b1p", bufs=2) as b1p,
                tc.tile_pool(name="h1gp", bufs=17) as hp,
                tc.tile_pool(name="ps1", bufs=3, space="PSUM") as pp1,
                tc.tile_pool(name="ps2", bufs=3, space="PSUM") as pp2,
            ):
                for e in range(E):
                    w1t = []
                    for hc in range(HC):
                        w = w1p.tile([P, F], F32R, tag="w1")
                        nc.sync.dma_start(w[:], w1_d[ds(e * H + hc * P, P), :])
                        w1t.append(w)
                    w2t = []
                    for fc in range(FC):
                        w = w2p.tile([P, H], F32R, tag="w2")
                        nc.sync.dma_start(w[:], w2_d[ds(e * F + fc * P, P), :])
                        w2t.append(w)
                    b1t = b1p.tile([P, FC], F32, tag="b1")
                    nc.sync.dma_start(b1t[:], b1_d[ds(e * P, P), :])

                    for t4 in range(TS4):
                        hts = []
                        for fc in range(FC):
                            p1 = pp1.tile([P, NT], F32, tag="p1")
                            for hc in range(HC):
                                nc.tensor.matmul(
                                    p1[:], w1t[hc][:, ts(fc, P)],
                                    xT[:, hc, ts(t4, NT)],
                                    start=(hc == 0), stop=(hc == HC - 1),
                                )
                            hg = hp.tile([P, NT], F32R, tag="h1g")
                            nc.scalar.activation(
                                hg[:], p1[:], ACT_FN,
                                bias=b1t[:, fc:fc + 1], scale=1.0,
                            )
                            hts.append(hg)
                        for tsub in range(NT // P):
                            tt = t4 * (NT // P) + tsub
                            p2 = pp2.tile([P, H], F32, tag="p2")
                            for fc in range(FC):
                                nc.tensor.matmul(
                                    p2[:], hts[fc][:, ts(tsub, P)], w2t[fc][:],
                                    start=(fc == 0), stop=(fc == FC - 1),
                                )
                            # out_acc[tt] += wsel[tt, e] * y
                            nc.vector.scalar_tensor_tensor(
                                out=out_acc[:, tt, :], in0=p2[:],
                                scalar=wsel[:, tt, e:e + 1],
                                in1=out_acc[:, tt, :],
                                op0=OP.mult, op1=OP.add,
                            )

            nc.sync.dma_start(out_d.rearrange("(c p) o -> p c o", p=P), out_acc[:])

    nc.compile()
    return nc


def _prep(inputs):
    xs = np.ascontiguousarray(np.asarray(inputs["x"], np.float32))
    wg = np.asarray(inputs["W_g"], np.float32).T[None]        # [1, E, H]
    wg = np.ascontiguousarray(np.broadcast_to(wg, (P, E, H))).reshape(P * E, H)
    w1 = np.ascontiguousarray(np.asarray(inputs["w1"], np.float32)).reshape(E * H, F)
    b1 = np.asarray(inputs["b1"], np.float32).reshape(E, FC, P)
    b1 = np.ascontiguousarray(b1.transpose(0, 2, 1)).reshape(E * P, FC)
    w2 = np.ascontiguousarray(np.asarray(inputs["w2"], np.float32)).reshape(E * F, H)
    b2 = np.ascontiguousarray(np.asarray(inputs["b2"], np.float32))
    return xs, wg, w1, b1, w2, b2


def kernel(trace=False, **inputs):
    from concourse.bass_utils import run_bass_kernel_spmd

    if "nc" not in _CACHE:
        _CACHE["nc"] = _build()
    nc = _CACHE["nc"]

    xs, wg, w1, b1, w2, b2 = _prep(inputs)
    in_maps = []
    for c in range(B):
        in_maps.append({
            "x": np.ascontiguousarray(xs[c]),
            "W_g": wg, "w1": w1, "b1": b1, "w2": w2, "b2": b2,
        })
    res = run_bass_kernel_spmd(nc, in_maps, core_ids=list(range(B)), trace=trace)
    out = np.stack([r["out"] for r in res.results], axis=0)
    if trace:
        return out, res
    return out


# revision 19
# speedup vs baseline: 1.0018x; 1.0018x over previous
"""MoE model (B=8,S=2048,H=512,E=8,K=2) on 8 TRN2 NeuronCores.

Strategy: data-parallel over the batch dim (one batch row of 2048 tokens per
core). Each core computes the gate (softmax + top-2 via full top-8 sort),
then iterates over all 8 experts densely: h = gelu(x @ w1[e] + b1[e]),
y = h @ w2[e], accumulating out += wsel[:, e] * y in SBUF, with the b2
contribution folded in as an init matmul out0 = wsel @ b2. All matmuls run
as float32r (full fp32 storage, ~bf16 PE throughput, ~1e-3 accuracy).

Layout notes:
 - xT (transposed x) is built on-device via PE transposes; it feeds both the
   gate matmul and layer-1 (contraction over H needs H on partitions).
 - Layer-1 output h1 is kept transposed [f, t] so layer-2 can use it directly
   as the stationary operand, producing y in natural [t, o] layout where the
   per-token gate weight is a per-partition scalar multiply.
"""

import numpy as np

B, S, H, E = 8, 2048, 512, 8
F = 4 * H          # 2048
T = S              # tokens per core (one batch row per core)
P = 128
HC = H // P        # 4 contraction chunks for layer 1
FC = F // P        # 16 f chunks
TT = T // P        # 16 token tiles
NT = 512           # moving-dim tile (max for 4-byte dtypes)
TS4 = T // NT      # 4 token chunks of 512

_CACHE = {}


def _build(act_name="Gelu"):
    from concourse import bacc
    import concourse.bass as bass
    import concourse.mybir as mybir
    import concourse.tile as tile
    from concourse.masks import make_identity

    ts = bass.ts
    ds = bass.ds
    F32 = mybir.dt.float32
    F32R = mybir.dt.float32r
    AF = mybir.ActivationFunctionType
    OP = mybir.AluOpType

    ACT_FN = getattr(mybir.ActivationFunctionType, act_name)

    nc = bacc.Bacc("TRN2", target_bir_lowering=False)

    x_d = nc.dram_tensor("x", [T, H], F32, kind="ExternalInput")
    wg_d = nc.dram_tensor("W_g", [P * E, H], F32, kind="ExternalInput")
    w1_d = nc.dram_tensor("w1", [E * H, F], F32R, kind="ExternalInput")
    b1_d = nc.dram_tensor("b1", [E * P, FC], F32, kind="ExternalInput")
    w2_d = nc.dram_tensor("w2", [E * F, H], F32R, kind="ExternalInput")
    b2_d = nc.dram_tensor("b2", [E, H], F32, kind="ExternalInput")
    out_d = nc.dram_tensor("out", [T, H], F32, kind="ExternalOutput")

    with tile.TileContext(nc) as tc:
        with tc.tile_pool(name="const", bufs=1) as cpool:
            ident_f = cpool.tile([P, P], F32)
            make_identity(nc, ident_f[:])

            b2sb = cpool.tile([E, H], F32)
            nc.sync.dma_start(b2sb[:], b2_d[:])

            xT = cpool.tile([P, HC, T], F32R)        # x transposed [h, t]
            wsel = cpool.tile([P, TT, E], F32)       # masked gate weights
            out_acc = cpool.tile([P, TT, H], F32)    # running output [t, o]

            # W_g columns broadcast across partitions (host-prepared layout):
            # wgb[p, e, :] = W_g[:, e] for every p. The gate runs as exact-fp32
            # dot products on DVE; the PE's fp32 modes carry ~1e-4 error,
            # enough to flip near-tied top-2 picks.
            wgb = cpool.tile([P, E, H], F32)
            nc.sync.dma_start(wgb[:], wg_d.rearrange("(p c) h -> p c h", p=P))

            # ---- stage 1: load x, transpose into xT, gate ---------------
            with (
                tc.tile_pool(name="xstage", bufs=3) as xpool,
                tc.tile_pool(name="tpsum", bufs=2, space="PSUM") as tpsum,
                tc.tile_pool(name="gate", bufs=4) as gpool,
                tc.tile_pool(name="gsc", bufs=3) as gsc,
                tc.tile_pool(name="wtpsum", bufs=2, space="PSUM") as wtpsum,
                tc.tile_pool(name="bpsum", bufs=2, space="PSUM") as bpsum,
            ):
                for tt in range(TT):
                    xt = xpool.tile([P, H], F32, tag="xt")
                    nc.sync.dma_start(xt[:], x_d[ts(tt, P), :])
                    for hc in range(HC):
                        ps = tpsum.tile([P, P], F32, tag="tp")
                        nc.tensor.transpose(ps[:], xt[:, ts(hc, P)], ident_f[:])
                        # cast-copy rounds to f32r for the expert matmuls
                        nc.vector.tensor_copy(xT[:, hc, ts(tt, P)], ps[:])

                    # gate logits: IEEE-fp32 products on GpSimd (the DVE
                    # fp32 multiplier is ~tf32 and flips near-tied top-2
                    # picks), exact reduce-add on DVE
                    logit = gpool.tile([P, E], F32, tag="logit")
                    for e in range(E):
                        sc = gsc.tile([P, H], F32, tag="gsc")
                        nc.gpsimd.tensor_tensor(
                            out=sc[:], in0=xt[:],
                            in1=wgb[:, e, :], op=OP.mult,
                        )
                        nc.vector.tensor_reduce(
                            out=logit[:, e:e + 1], in_=sc[:],
                            axis=mybir.AxisListType.X, op=OP.add,
                        )
                    srt = gpool.tile([P, 8], F32, tag="srt")
                    nc.vector.max(srt[:], logit[:])
                    nmax = gpool.tile([P, 1], F32, tag="nmax")
                    nc.vector.tensor_scalar_mul(nmax[:], srt[:, 0:1], -1.0)
                    expv = gpool.tile([P, E], F32, tag="expv")
                    sume = gpool.tile([P, 1], F32, tag="sume")
                    nc.scalar.activation(
                        expv[:], logit[:], AF.Exp,
                        bias=nmax[:], scale=1.0, accum_out=sume[:],
                    )
                    rsum = gpool.tile([P, 1], F32, tag="rsum")
                    nc.vector.reciprocal(rsum[:], sume[:])
                    probs = gpool.tile([P, E], F32, tag="probs")
                    nc.vector.tensor_scalar_mul(probs[:], expv[:], rsum[:])
                    # wsel = (logit >= 2nd-max) * probs  -> top-2 weights
                    nc.vector.scalar_tensor_tensor(
                        out=wsel[:, tt, :], in0=logit[:], scalar=srt[:, 1:2],
                        in1=probs[:], op0=OP.is_ge, op1=OP.mult,
                    )
                    # out_acc[tt] = wsel[tt] @ b2  (the weighted-b2 term)
                    wtp = wtpsum.tile([E, P], F32, tag="wtp")
                    nc.tensor.transpose(wtp[:], wsel[:, tt, :], ident_f[:])
                    wts = gpool.tile([E, P], F32, tag="wts")
                    nc.vector.tensor_copy(wts[:], wtp[:])
                    bp = bpsum.tile([P, H], F32, tag="bp")
                    nc.tensor.matmul(bp[:], wts[:], b2sb[:], start=True, stop=True)
                    nc.vector.tensor_copy(out_acc[:, tt, :], bp[:])

            # ---- stage 3: experts --------------------------------------
            with (
                tc.tile_pool(name="w1p", bufs=5) as w1p,
                tc.tile_pool(name="w2p", bufs=18) as w2p,
                tc.tile_pool(name="b1p", bufs=2) as b1p,
                tc.tile_pool(name="h1gp", bufs=17) as hp,
                tc.tile_pool(name="ps1", bufs=3, space="PSUM") as pp1,
                tc.tile_pool(name="ps2", bufs=3, space="PSUM") as pp2,
            ):
                for e in range(E):
                    w1t = []
                    for hc in range(HC):
                        w = w1p.tile([P, F], F32R, tag="w1")
                        nc.sync.dma_start(w[:], w1_d[ds(e * H + hc * P, P), :])
                        w1t.append(w)
                    w2t = []
                    for fc in range(FC):
                        w = w2p.tile([P, H], F32R, tag="w2")
                        nc.sync.dma_start(w[:], w2_d[ds(e * F + fc * P, P), :])
                        w2t.append(w)
                    b1t = b1p.tile([P, FC], F32, tag="b1")
                    nc.sync.dma_start(b1t[:], b1_d[ds(e * P, P), :])

                    for t4 in range(TS4):
                        hts = []
                        for fc in range(FC):
                            p1 = pp1.tile([P, NT], F32, tag="p1")
                            for hc in range(HC):
                                nc.tensor.matmul(
                                    p1[:], w1t[hc][:, ts(fc, P)],
                                    xT[:, hc, ts(t4, NT)],
                                    start=(hc == 0), stop=(hc == HC - 1),
                                )
                            hg = hp.tile([P, NT], F32R, tag="h1g")
                            nc.scalar.activation(
                                hg[:], p1[:], ACT_FN,
                                bias=b1t[:, fc:fc + 1], scale=1.0,
                            )
                            hts.append(hg)
                        for tsub in range(NT // P):
                            tt = t4 * (NT // P) + tsub
                            p2 = pp2.tile([P, H], F32, tag="p2")
                            for fc in range(FC):
                                nc.tensor.matmul(
                                    p2[:], hts[fc][:, ts(tsub, P)], w2t[fc][:],
                                    start=(fc == 0), stop=(fc == FC - 1),
                                )
                            # out_acc[tt] += wsel[tt, e] * y
                            nc.vector.scalar_tensor_tensor(
                                out=out_acc[:, tt, :], in0=p2[:],
                                scalar=wsel[:, tt, e:e + 1],
                                in1=out_acc[:, tt, :],
                                op0=OP.mult, op1=OP.add,
                            )

            nc.sync.dma_start(out_d.rearrange("(c p) o -> p c o", p=P), out_acc[:])

    nc.compile()
    return nc


def _prep(inputs):
    xs = np.ascontiguousarray(np.asarray(inputs["x"], np.float32))
    wg = np.asarray(inputs["W_g"], np.float32).T[None]        # [1, E, H]
    wg = np.ascontiguousarray(np.broadcast_to(wg, (P, E, H))).reshape(P * E, H)
    w1 = np.ascontiguousarray(np.asarray(inputs["w1"], np.float32)).reshape(E * H, F)
    b1 = np.asarray(inputs["b1"], np.float32).reshape(E, FC, P)
    b1 = np.ascontiguousarray(b1.transpose(0, 2, 1)).reshape(E * P, FC)
    w2 = np.ascontiguousarray(np.asarray(inputs["w2"], np.float32)).reshape(E * F, H)
    b2 = np.ascontiguousarray(np.asarray(inputs["b2"], np.float32))
    return xs, wg, w1, b1, w2, b2


def kernel(trace=False, **inputs):
    from concourse.bass_utils import run_bass_kernel_spmd

    if "nc" not in _CACHE:
        _CACHE["nc"] = _build()
    nc = _CACHE["nc"]

    xs, wg, w1, b1, w2, b2 = _prep(inputs)
    in_maps = []
    for c in range(B):
        in_maps.append({
            "x": np.ascontiguousarray(xs[c]),
            "W_g": wg, "w1": w1, "b1": b1, "w2": w2, "b2": b2,
        })
    res = run_bass_kernel_spmd(nc, in_maps, core_ids=list(range(B)), trace=trace)
    out = np.stack([r["out"] for r in res.results], axis=0)
    if trace:
        return out, res
    return out
